# revision 35
# baseline (speedup 1.0000x reference)
"""Distributed attention layer kernel for 8 TRN2 NeuronCores.

Reference computation (f32):
    Q = q @ W_q; K = k @ W_k; V = v @ W_v
    out = softmax((Q @ K^T)/sqrt(d_k)) @ V

Sharding: rows of q/k/v are split 8 ways (sequence parallel). Each core
projects its own shards, the K^T/V projections are all-gathered (fp16),
and each core computes its 512-row slice of the attention output.

Precision: projections run in f32r (fp32 operands, PE rounds mantissas
to 11 bits, full rate for free-dim >= 256) with f32 PSUM accumulation.
K^T/Q^T/V are downcast to fp16 for the attention matmuls (QK^T and PV
single plain fp16 matmuls, f32 accumulation). Softmax is f32 (ACT exp
with per-row max bias, fused row-sum). Measured end-to-end error vs the
f32 reference: ~8e-3 (gate 2e-2).
"""

import os
import sys

for _p in ("/opt/pypackages", "/opt/trn_rl_repo"):
    if _p not in sys.path:
        sys.path.insert(0, _p)

import numpy as np

N_Q, N_KV, DIM = 4096, 4096, 1024  # D_K = D_V = DIM (square weights)
CORES = 8

P = 128


def build_attention(nq=N_Q, dim=DIM, cores=CORES):
    """Build the per-core Bass graph (SPMD; identical on all cores)."""
    import concourse.bass as bass
    import concourse.mybir as mybir
    from concourse import bacc
    from concourse.masks import make_identity
    from concourse.tile import TileContext

    dt = mybir.dt
    f32, f32r, f16 = dt.float32, dt.float32r, dt.float16

    sh = nq // cores          # rows per core (512)
    n_ct = dim // P           # contraction tiles for projections (8)
    n_dt = dim // P           # d tiles (8)
    n_it = sh // P            # query-row tiles per core (4)
    n_jjt = sh // P           # kv-row tiles per core (4)
    n_jt = nq // P            # total kv j tiles (32)
    JG = 4                    # j-tiles per PV V-chunk
    n_jg = n_jt // JG         # V chunk count (8)
    EH = 512
    n_eh = dim // EH          # 512-wide output column halves (2)
    hd = dim // 2
    nh = n_dt // 2
    scale = 1.0 / float(np.sqrt(dim))

    nc = bacc.Bacc(num_devices=cores)

    # --- external I/O (per core: row shards of q/k/v, full weights) ---
    q_ext = nc.declare_dram_parameter("q", [sh, dim], f32, isOutput=False)
    k_ext = nc.declare_dram_parameter("k", [sh, dim], f32, isOutput=False)
    v_ext = nc.declare_dram_parameter("v", [sh, dim], f32, isOutput=False)
    wq_ext = nc.declare_dram_parameter("W_q", [dim, dim], f32r, isOutput=False)
    wk_ext = nc.declare_dram_parameter("W_k", [dim, dim], f32r, isOutput=False)
    wv_ext = nc.declare_dram_parameter("W_v", [dim, dim], f32r, isOutput=False)
    out_ext = nc.declare_dram_parameter("out", [sh, dim], f32, isOutput=True)

    # --- internal DRAM for collectives ---
    bounce_k = nc.dram_tensor("bounce_k", [dim, sh], f16)
    bounce_v = nc.dram_tensor("bounce_v", [sh, dim], f16)
    gath_k = nc.dram_tensor("gath_k", [cores * dim, sh], f16, addr_space="Shared")
    gath_v = nc.dram_tensor("gath_v", [cores * sh, dim], f16, addr_space="Shared")

    rg = [list(range(cores))]

    with TileContext(nc) as tc:
        with (
            tc.tile_pool(name="const", bufs=1) as constp,
            tc.tile_pool(name="qt", bufs=1) as qtp,
            tc.tile_pool(name="stats", bufs=1) as statp,
        ):
            # NOTE: make_identity/PE-transpose on float32r crashes walrus
            # codegen; transposes run in plain f32 and the psum result is
            # copy-cast (bit-identical) into float32r SBUF tiles.
            ident_f = constp.tile([P, P], f32, tag="idf", name="idf")
            make_identity(nc, ident_f)

            qthi = qtp.tile([P, n_dt, sh], f16, tag="qthi", name="qthi")
            # v_loc outlives the projection pools: its bounce DMA is issued
            # mid-S-phase to delay the V all-gather until the khi chunk
            # loads have drained (avoids DRAM contention with the gather)
            v_loc = qtp.tile([P, sh // P, dim], f16, tag="v_loc", name="v_loc")

            with (
                tc.tile_pool(name="w", bufs=1) as wpool,
                tc.tile_pool(name="iost", bufs=6) as iost,
                tc.tile_pool(name="tin", bufs=2) as tpool,
                tc.tile_pool(name="kvout", bufs=1) as kvout,
                tc.tile_pool(name="tpsum", bufs=4, space="PSUM") as tpsum,
                tc.tile_pool(name="ppsum", bufs=2, space="PSUM") as ppsum,
            ):
                # All bulk loads (inputs + weights) stream in order on the
                # sync (SP) HWDGE queue; the Activation HWDGE queue is kept
                # for small latency-critical transfers (bounce buffers, P^T
                # XBAR transposes, outputs) so their triggers never stall the
                # ACT engine behind megabytes of weight traffic.
                def load_input(x_ext):
                    stgs = []
                    xsrc = x_ext.rearrange("(it p) c -> p it c", p=P)
                    for it in range(sh // P):
                        stg = iost.tile([P, dim], f32, tag="iostg", name="iostg")
                        nc.sync.dma_start(stg[:], xsrc[:, it])
                        stgs.append(stg)
                    return stgs

                wk = wpool.tile([P, n_ct, dim], f32r, tag="wk", name="wk")
                wq = wpool.tile([P, n_ct, dim], f32r, tag="wq", name="wq")
                wv = wpool.tile([P, n_ct, dim], f32r, tag="wv", name="wv")
                wk_src = wk_ext.rearrange("(ct p) d -> p ct d", p=P)
                wq_src = wq_ext.rearrange("(ct p) d -> p ct d", p=P)
                wv_src = wv_ext.rearrange("(ct p) d -> p ct d", p=P)

                k_stg = load_input(k_ext)
                nc.sync.dma_start(wk[:, :, :hd], wk_src[:, :, :hd])
                nc.sync.dma_start(wk[:, :, hd:], wk_src[:, :, hd:])
                q_stg = load_input(q_ext)
                nc.sync.dma_start(wq[:, :, :hd], wq_src[:, :, :hd])
                nc.sync.dma_start(wq[:, :, hd:], wq_src[:, :, hd:])
                nc.sync.dma_start(wv[:, :, :hd], wv_src[:, :, :hd])
                nc.sync.dma_start(wv[:, :, hd:], wv_src[:, :, hd:])

                def transpose_input(stgs, tag):
                    """Transpose a staged [sh, dim] f32 input on the PE into a
                    [c_in=128, ct, row] f32r SBUF tile (copy-cast from psum)."""
                    xt = tpool.tile([P, n_ct, sh], f32r, tag=tag, name=tag)
                    for it, stg in enumerate(stgs):
                        dst = slice(it * P, (it + 1) * P)
                        for ct in range(n_ct):
                            ps = tpsum.tile([P, P], f32, tag="tps", name="tps")
                            nc.tensor.transpose(
                                ps[:], stg[:, ct * P:(ct + 1) * P], ident_f
                            )
                            nc.vector.tensor_copy(xt[:, ct, dst], ps[:])
                    return xt

                # ---- K path first: project K^T, bounce out, all-gather.
                # Single gather: the kernel-entry CC barrier (~45-55us of
                # launch skew) gates the first collective anyway, and Shared
                # DRAM reads starve while any collective is active, so one
                # gather followed by a full-speed khi prefetch beats split
                # gathers whose chunk reads crawl under the second one. ----
                kt = transpose_input(k_stg, "xt")
                kt_loc = kvout.tile([P, n_dt, sh], f16, tag="kt_loc", name="kt_loc")
                bk = bounce_k.rearrange("(dtt p) jj -> p dtt jj", p=P)
                for dtt in range(n_dt):
                    ps = ppsum.tile([P, sh], f32, tag="pps", name="pps")
                    dsl = slice(dtt * P, (dtt + 1) * P)
                    for ct in range(n_ct):
                        nc.tensor.matmul(
                            ps[:], wk[:, ct, dsl], kt[:, ct],
                            start=(ct == 0), stop=(ct == n_ct - 1),
                        )
                    nc.scalar.copy(kt_loc[:, dtt], ps[:])
                nc.scalar.dma_start(bk[:], kt_loc[:])
                nc.gpsimd.collective_compute(
                    "AllGather", mybir.AluOpType.bypass, replica_groups=rg,
                    ins=[bounce_k.ap().opt()], outs=[gath_k.ap().opt()],
                )

                # ---- Q path (local only): project Q^T, downcast to fp16 ----
                qt = transpose_input(q_stg, "xt")
                for dtt in range(n_dt):
                    ps = ppsum.tile([P, sh], f32, tag="pps", name="pps")
                    dsl = slice(dtt * P, (dtt + 1) * P)
                    for ct in range(n_ct):
                        nc.tensor.matmul(
                            ps[:], wq[:, ct, dsl], qt[:, ct],
                            start=(ct == 0), stop=(ct == n_ct - 1),
                        )
                    nc.scalar.copy(qthi[:, dtt], ps[:])

                # ---- V path: project V shard, downcast ----
                v_stg = load_input(v_ext)
                vt = transpose_input(v_stg, "xt")
                for jjt in range(n_jjt):
                    jsl = slice(jjt * P, (jjt + 1) * P)
                    for eh in range(n_eh):
                        ps = ppsum.tile([P, EH], f32, tag="ppsv", name="ppsv")
                        esl = slice(eh * EH, (eh + 1) * EH)
                        for ct in range(n_ct):
                            nc.tensor.matmul(
                                ps[:], vt[:, ct, jsl], wv[:, ct, esl],
                                start=(ct == 0), stop=(ct == n_ct - 1),
                            )
                        nc.scalar.copy(v_loc[:, jjt, esl], ps[:])

            # ================= attention phase =================
            m_t = [statp.tile([P, 1], f32, tag=f"m{it}", name=f"m{it}") for it in range(n_it)]
            tmpmax = statp.tile([P, 1], f32, tag="tmpmax", name="tmpmax")
            bias_t = [statp.tile([P, 1], f32, tag=f"b{it}", name=f"b{it}") for it in range(n_it)]
            ell_t = [statp.tile([P, 1], f32, tag=f"l{it}", name=f"l{it}") for it in range(n_it)]
            rl_t = [statp.tile([P, 1], f32, tag=f"r{it}", name=f"r{it}") for it in range(n_it)]

            gk = gath_k.rearrange("(r dtt p) jj -> r p dtt jj", r=cores, p=P)
            gv = gath_v.rearrange("(jg jj p) e -> jg p jj e", jj=JG, p=P)
            bv = bounce_v.rearrange("(jjt p) e -> p jjt e", p=P)

            with (
                tc.tile_pool(name="schunk", bufs=5) as schunk,
                tc.tile_pool(name="srow", bufs=n_it) as srow,
                tc.tile_pool(name="prow", bufs=2) as prow,
                tc.tile_pool(name="ptp", bufs=1) as ptp,
                tc.tile_pool(name="vchunk", bufs=3) as vchunk,
                tc.tile_pool(name="opool", bufs=2) as opool,
            ):
                s_sb = [srow.tile([P, nq], f32, tag="s", name="s") for _ in range(n_it)]

                # ---- scores: all khi chunk loads issued upfront (they
                # stream at full bandwidth in the collective-free window
                # right after the K gather), then S with running row max.
                # The V gather is released only after rr==2 so it does not
                # starve the tail of the khi prefetch. ----
                khis = []
                for rr in range(cores):
                    khi = schunk.tile([P, n_dt, sh], f16, tag="khi", name="khi")
                    nc.sync.dma_start(khi[:], gk[rr])
                    khis.append(khi)

                p_sb = [prow.tile([P, nq], f16, tag="p", name="p") for _ in range(n_it)]
                pt = [
                    ptp.tile([P, n_jt, P], f16, tag=f"pt{it}", name=f"pt{it}")
                    for it in range(n_it)
                ]

                _spsum_cm = tc.tile_pool(name="spsum", bufs=6, space="PSUM")
                spsum = _spsum_cm.__enter__()
                for rr in range(cores):
                    rsl = slice(rr * sh, (rr + 1) * sh)
                    for it in range(n_it):
                        isl = slice(it * P, (it + 1) * P)
                        ps = spsum.tile([P, sh], f32, tag="sps", name="sps")
                        for dtt in range(n_dt):
                            nc.tensor.matmul(
                                ps[:], qthi[:, dtt, isl], khis[rr][:, dtt],
                                start=(dtt == 0), stop=(dtt == n_dt - 1),
                            )
                        if rr == 0:
                            nc.vector.reduce_max(
                                m_t[it][:], ps[:], axis=mybir.AxisListType.X
                            )
                        else:
                            nc.vector.reduce_max(
                                tmpmax[:], ps[:], axis=mybir.AxisListType.X
                            )
                            nc.vector.tensor_max(m_t[it][:], m_t[it][:], tmpmax[:])
                        nc.scalar.copy(s_sb[it][:, rsl], ps[:])
                        if rr == cores - 1:
                            # softmax fires per row tile as soon as its last
                            # chunk lands: exp(it) on ACT and the P^T XBAR
                            # transpose overlap the remaining S matmuls
                            nc.vector.tensor_scalar_mul(
                                bias_t[it][:], m_t[it][:], -scale
                            )
                            nc.scalar.activation(
                                p_sb[it][:], s_sb[it][:],
                                mybir.ActivationFunctionType.Exp,
                                bias=bias_t[it][:], scale=scale,
                                accum_out=ell_t[it][:],
                            )
                            nc.vector.reciprocal(rl_t[it][:], ell_t[it][:])
                            nc.scalar.dma_start_transpose(pt[it][:], p_sb[it][:])
                    if rr == 2:
                        # bounce rides the scalar queue behind rr<=2's copies,
                        # so the V gather starts only once the khi prefetch
                        # has drained; gpsimd emission stays after the K
                        # collective so khi loads never wait on its tick.
                        nc.scalar.dma_start(bv[:], v_loc[:])
                        nc.gpsimd.collective_compute(
                            "AllGather", mybir.AluOpType.bypass, replica_groups=rg,
                            ins=[bounce_v.ap().opt()], outs=[gath_v.ap().opt()],
                        )
                _spsum_cm.__exit__(None, None, None)

                # ---- O = (P @ V) / ell, all 8 PSUM banks, single V pass ----
                _pvpsum_cm = tc.tile_pool(name="pvpsum", bufs=n_it * n_eh, space="PSUM")
                pvpsum = _pvpsum_cm.__enter__()
                pso = {
                    (it, eh): pvpsum.tile([P, EH], f32, tag="pvps", name="pvps")
                    for it in range(n_it) for eh in range(n_eh)
                }
                for jg in range(n_jg):
                    vc = vchunk.tile([P, JG, dim], f16, tag="vc", name="vc")
                    nc.sync.dma_start(vc[:], gv[jg])
                    last = jg == n_jg - 1
                    for it in range(n_it):
                        for eh in range(n_eh):
                            esl = slice(eh * EH, (eh + 1) * EH)
                            for jj in range(JG):
                                nc.tensor.matmul(
                                    pso[(it, eh)][:],
                                    pt[it][:, jg * JG + jj],
                                    vc[:, jj, esl],
                                    start=(jg == 0 and jj == 0),
                                    stop=(last and jj == JG - 1),
                                )
                        if last:
                            # scale + store this row tile while the PE is
                            # still accumulating the remaining row tiles
                            o_sb = opool.tile([P, dim], f32, tag="o", name="o")
                            for eh in range(n_eh):
                                esl = slice(eh * EH, (eh + 1) * EH)
                                nc.vector.tensor_scalar_mul(
                                    o_sb[:, esl], pso[(it, eh)][:], rl_t[it][:]
                                )
                            nc.scalar.dma_start(
                                out_ext[it * P:(it + 1) * P, :], o_sb[:]
                            )
                _pvpsum_cm.__exit__(None, None, None)

    return nc


_CACHE = {}
RUN_KW = {}


def _get_nc():
    if "nc" not in _CACHE:
        _CACHE["nc"] = build_attention()
    return _CACHE["nc"]


def kernel(**inputs):
    from concourse.bass_utils import run_bass_kernel_spmd

    q = np.ascontiguousarray(np.asarray(inputs["q"], dtype=np.float32))
    k = np.ascontiguousarray(np.asarray(inputs["k"], dtype=np.float32))
    v = np.ascontiguousarray(np.asarray(inputs["v"], dtype=np.float32))
    W_q = np.ascontiguousarray(np.asarray(inputs["W_q"], dtype=np.float32))
    W_k = np.ascontiguousarray(np.asarray(inputs["W_k"], dtype=np.float32))
    W_v = np.ascontiguousarray(np.asarray(inputs["W_v"], dtype=np.float32))

    sh = N_Q // CORES
    in_maps = []
    for r in range(CORES):
        sl = slice(r * sh, (r + 1) * sh)
        in_maps.append({
            "q": q[sl], "k": k[sl], "v": v[sl],
            "W_q": W_q, "W_k": W_k, "W_v": W_v,
        })

    nc = _get_nc()
    if not nc.is_finalized():
        nc.finalize()
    res = run_bass_kernel_spmd(nc, in_maps, core_ids=list(range(CORES)), **RUN_KW)
    _CACHE["last_result"] = res
    out = np.concatenate([res.results[r]["out"] for r in range(CORES)], axis=0)
    return out


if __name__ == "__main__":
    import reference

    inputs = {kk: np.asarray(vv) for kk, vv in reference.setup_inputs().items()}
    out = kernel(**inputs)
    print("out shape:", out.shape, out.dtype)


# revision 36
# speedup vs baseline: 1.0699x; 1.0699x over previous
"""Distributed attention layer kernel for 8 TRN2 NeuronCores.

Reference computation (f32):
    Q = q @ W_q; K = k @ W_k; V = v @ W_v
    out = softmax((Q @ K^T)/sqrt(d_k)) @ V

Sharding: rows of q/k/v are split 8 ways (sequence parallel). Each core
projects its own shards, the K^T/V projections are all-gathered (fp16),
and each core computes its 512-row slice of the attention output.

Precision: projections run in f32r (fp32 operands, PE rounds mantissas
to 11 bits, full rate for free-dim >= 256) with f32 PSUM accumulation.
K^T/Q^T/V are downcast to fp16 for the attention matmuls (QK^T and PV
single plain fp16 matmuls, f32 accumulation). Softmax is f32 (ACT exp
with per-row max bias, fused row-sum). Measured end-to-end error vs the
f32 reference: ~8e-3 (gate 2e-2).
"""

import os
import sys

for _p in ("/opt/pypackages", "/opt/trn_rl_repo"):
    if _p not in sys.path:
        sys.path.insert(0, _p)

import numpy as np

N_Q, N_KV, DIM = 4096, 4096, 1024  # D_K = D_V = DIM (square weights)
CORES = 8

P = 128


def build_attention(nq=N_Q, dim=DIM, cores=CORES):
    """Build the per-core Bass graph (SPMD; identical on all cores)."""
    import concourse.bass as bass
    import concourse.mybir as mybir
    from concourse import bacc
    from concourse.masks import make_identity
    from concourse.tile import TileContext

    dt = mybir.dt
    f32, f32r, f16 = dt.float32, dt.float32r, dt.float16

    sh = nq // cores          # rows per core (512)
    n_ct = dim // P           # contraction tiles for projections (8)
    n_dt = dim // P           # d tiles (8)
    n_it = sh // P            # query-row tiles per core (4)
    n_jjt = sh // P           # kv-row tiles per core (4)
    n_jt = nq // P            # total kv j tiles (32)
    JG = 4                    # j-tiles per PV V-chunk
    n_jg = n_jt // JG         # V chunk count (8)
    EH = 512
    n_eh = dim // EH          # 512-wide output column halves (2)
    hd = dim // 2
    nh = n_dt // 2
    scale = 1.0 / float(np.sqrt(dim))

    nc = bacc.Bacc(num_devices=cores)

    # --- external I/O (per core: row shards of q/k/v, full weights) ---
    q_ext = nc.declare_dram_parameter("q", [sh, dim], f32, isOutput=False)
    k_ext = nc.declare_dram_parameter("k", [sh, dim], f32, isOutput=False)
    v_ext = nc.declare_dram_parameter("v", [sh, dim], f32, isOutput=False)
    wq_ext = nc.declare_dram_parameter("W_q", [dim, dim], f32r, isOutput=False)
    wk_ext = nc.declare_dram_parameter("W_k", [dim, dim], f32r, isOutput=False)
    wv_ext = nc.declare_dram_parameter("W_v", [dim, dim], f32r, isOutput=False)
    out_ext = nc.declare_dram_parameter("out", [sh, dim], f32, isOutput=True)

    # --- internal DRAM for collectives ---
    bounce_k = nc.dram_tensor("bounce_k", [dim, sh], f16)
    bounce_v = nc.dram_tensor("bounce_v", [sh, dim], f16)
    gath_k = nc.dram_tensor("gath_k", [cores * dim, sh], f16, addr_space="Shared")
    gath_v = nc.dram_tensor("gath_v", [cores * sh, dim], f16, addr_space="Shared")

    rg = [list(range(cores))]

    with TileContext(nc) as tc:
        with (
            tc.tile_pool(name="const", bufs=1) as constp,
            tc.tile_pool(name="qt", bufs=1) as qtp,
            tc.tile_pool(name="stats", bufs=1) as statp,
        ):
            # NOTE: make_identity/PE-transpose on float32r crashes walrus
            # codegen; transposes run in plain f32 and the psum result is
            # copy-cast (bit-identical) into float32r SBUF tiles.
            ident_f = constp.tile([P, P], f32, tag="idf", name="idf")
            make_identity(nc, ident_f)

            qthi = qtp.tile([P, n_dt, sh], f16, tag="qthi", name="qthi")
            # v_loc outlives the projection pools: its bounce DMA is issued
            # mid-S-phase to delay the V all-gather until the khi chunk
            # loads have drained (avoids DRAM contention with the gather)
            v_loc = qtp.tile([P, sh // P, dim], f16, tag="v_loc", name="v_loc")

            with (
                tc.tile_pool(name="w", bufs=1) as wpool,
                tc.tile_pool(name="iost", bufs=6) as iost,
                tc.tile_pool(name="tin", bufs=2) as tpool,
                tc.tile_pool(name="kvout", bufs=1) as kvout,
                tc.tile_pool(name="tpsum", bufs=4, space="PSUM") as tpsum,
                tc.tile_pool(name="ppsum", bufs=2, space="PSUM") as ppsum,
            ):
                # All bulk loads (inputs + weights) stream in order on the
                # sync (SP) HWDGE queue; the Activation HWDGE queue is kept
                # for small latency-critical transfers (bounce buffers, P^T
                # XBAR transposes, outputs) so their triggers never stall the
                # ACT engine behind megabytes of weight traffic.
                def load_input(x_ext):
                    stgs = []
                    xsrc = x_ext.rearrange("(it p) c -> p it c", p=P)
                    for it in range(sh // P):
                        stg = iost.tile([P, dim], f32, tag="iostg", name="iostg")
                        nc.sync.dma_start(stg[:], xsrc[:, it])
                        stgs.append(stg)
                    return stgs

                wk = wpool.tile([P, n_ct, dim], f32r, tag="wk", name="wk")
                wq = wpool.tile([P, n_ct, dim], f32r, tag="wq", name="wq")
                wv = wpool.tile([P, n_ct, dim], f32r, tag="wv", name="wv")
                wk_src = wk_ext.rearrange("(ct p) d -> p ct d", p=P)
                wq_src = wq_ext.rearrange("(ct p) d -> p ct d", p=P)
                wv_src = wv_ext.rearrange("(ct p) d -> p ct d", p=P)

                k_stg = load_input(k_ext)
                nc.sync.dma_start(wk[:, :, :hd], wk_src[:, :, :hd])
                nc.sync.dma_start(wk[:, :, hd:], wk_src[:, :, hd:])
                q_stg = load_input(q_ext)
                nc.sync.dma_start(wq[:, :, :hd], wq_src[:, :, :hd])
                nc.sync.dma_start(wq[:, :, hd:], wq_src[:, :, hd:])
                nc.sync.dma_start(wv[:, :, :hd], wv_src[:, :, :hd])
                nc.sync.dma_start(wv[:, :, hd:], wv_src[:, :, hd:])

                def transpose_input(stgs, tag):
                    """Transpose a staged [sh, dim] f32 input on the PE into a
                    [c_in=128, ct, row] f32r SBUF tile (copy-cast from psum)."""
                    xt = tpool.tile([P, n_ct, sh], f32r, tag=tag, name=tag)
                    for it, stg in enumerate(stgs):
                        dst = slice(it * P, (it + 1) * P)
                        for ct in range(n_ct):
                            ps = tpsum.tile([P, P], f32, tag="tps", name="tps")
                            nc.tensor.transpose(
                                ps[:], stg[:, ct * P:(ct + 1) * P], ident_f
                            )
                            nc.vector.tensor_copy(xt[:, ct, dst], ps[:])
                    return xt

                # ---- K path first: project K^T, bounce out, all-gather.
                # Single gather: the kernel-entry CC barrier (~45-55us of
                # launch skew) gates the first collective anyway, and Shared
                # DRAM reads starve while any collective is active, so one
                # gather followed by a full-speed khi prefetch beats split
                # gathers whose chunk reads crawl under the second one. ----
                kt = transpose_input(k_stg, "xt")
                kt_loc = kvout.tile([P, n_dt, sh], f16, tag="kt_loc", name="kt_loc")
                bk = bounce_k.rearrange("(dtt p) jj -> p dtt jj", p=P)
                for dtt in range(n_dt):
                    ps = ppsum.tile([P, sh], f32, tag="pps", name="pps")
                    dsl = slice(dtt * P, (dtt + 1) * P)
                    for ct in range(n_ct):
                        nc.tensor.matmul(
                            ps[:], wk[:, ct, dsl], kt[:, ct],
                            start=(ct == 0), stop=(ct == n_ct - 1),
                        )
                    nc.scalar.copy(kt_loc[:, dtt], ps[:])
                nc.scalar.dma_start(bk[:], kt_loc[:])
                nc.gpsimd.collective_compute(
                    "AllGather", mybir.AluOpType.bypass, replica_groups=rg,
                    ins=[bounce_k.ap().opt()], outs=[gath_k.ap().opt()],
                )

                # ---- Q path (local only): project Q^T, downcast to fp16 ----
                qt = transpose_input(q_stg, "xt")
                for dtt in range(n_dt):
                    ps = ppsum.tile([P, sh], f32, tag="pps", name="pps")
                    dsl = slice(dtt * P, (dtt + 1) * P)
                    for ct in range(n_ct):
                        nc.tensor.matmul(
                            ps[:], wq[:, ct, dsl], qt[:, ct],
                            start=(ct == 0), stop=(ct == n_ct - 1),
                        )
                    nc.scalar.copy(qthi[:, dtt], ps[:])

                # ---- V path: project V shard, downcast ----
                v_stg = load_input(v_ext)
                vt = transpose_input(v_stg, "xt")
                for jjt in range(n_jjt):
                    jsl = slice(jjt * P, (jjt + 1) * P)
                    for eh in range(n_eh):
                        ps = ppsum.tile([P, EH], f32, tag="ppsv", name="ppsv")
                        esl = slice(eh * EH, (eh + 1) * EH)
                        for ct in range(n_ct):
                            nc.tensor.matmul(
                                ps[:], vt[:, ct, jsl], wv[:, ct, esl],
                                start=(ct == 0), stop=(ct == n_ct - 1),
                            )
                        nc.scalar.copy(v_loc[:, jjt, esl], ps[:])

            # ================= attention phase =================
            m_t = [statp.tile([P, 1], f32, tag=f"m{it}", name=f"m{it}") for it in range(n_it)]
            tmpmax = statp.tile([P, 1], f32, tag="tmpmax", name="tmpmax")
            bias_t = [statp.tile([P, 1], f32, tag=f"b{it}", name=f"b{it}") for it in range(n_it)]
            ell_t = [statp.tile([P, 1], f32, tag=f"l{it}", name=f"l{it}") for it in range(n_it)]
            rl_t = [statp.tile([P, 1], f32, tag=f"r{it}", name=f"r{it}") for it in range(n_it)]

            gk = gath_k.rearrange("(r dtt p) jj -> r p dtt jj", r=cores, p=P)
            gv = gath_v.rearrange("(jg jj p) e -> jg p jj e", jj=JG, p=P)
            bv = bounce_v.rearrange("(jjt p) e -> p jjt e", p=P)

            with (
                tc.tile_pool(name="schunk", bufs=5) as schunk,
                tc.tile_pool(name="srow", bufs=n_it) as srow,
                tc.tile_pool(name="prow", bufs=2) as prow,
                tc.tile_pool(name="ptp", bufs=1) as ptp,
                tc.tile_pool(name="vchunk", bufs=3) as vchunk,
                tc.tile_pool(name="opool", bufs=2) as opool,
            ):
                s_sb = [srow.tile([P, nq], f32, tag="s", name="s") for _ in range(n_it)]

                # ---- scores: all khi chunk loads issued upfront (they
                # stream at full bandwidth in the collective-free window
                # right after the K gather), then S with running row max.
                # The V gather is released only after rr==2 so it does not
                # starve the tail of the khi prefetch. ----
                khis = []
                for rr in range(cores):
                    khi = schunk.tile([P, n_dt, sh], f16, tag="khi", name="khi")
                    nc.sync.dma_start(khi[:], gk[rr])
                    khis.append(khi)

                p_sb = [prow.tile([P, nq], f16, tag="p", name="p") for _ in range(n_it)]
                pt = [
                    ptp.tile([P, n_jt, P], f16, tag=f"pt{it}", name=f"pt{it}")
                    for it in range(n_it)
                ]

                _spsum_cm = tc.tile_pool(name="spsum", bufs=6, space="PSUM")
                spsum = _spsum_cm.__enter__()
                for rr in range(cores):
                    rsl = slice(rr * sh, (rr + 1) * sh)
                    for it in range(n_it):
                        isl = slice(it * P, (it + 1) * P)
                        ps = spsum.tile([P, sh], f32, tag="sps", name="sps")
                        for dtt in range(n_dt):
                            nc.tensor.matmul(
                                ps[:], qthi[:, dtt, isl], khis[rr][:, dtt],
                                start=(dtt == 0), stop=(dtt == n_dt - 1),
                            )
                        if rr == 0:
                            nc.vector.reduce_max(
                                m_t[it][:], ps[:], axis=mybir.AxisListType.X
                            )
                        else:
                            nc.vector.reduce_max(
                                tmpmax[:], ps[:], axis=mybir.AxisListType.X
                            )
                            nc.vector.tensor_max(m_t[it][:], m_t[it][:], tmpmax[:])
                        if rr < cores - 1:
                            nc.scalar.copy(s_sb[it][:, rsl], ps[:])
                        else:
                            # last chunk's copies go on the vector engine so
                            # the inline exps below don't delay them (they
                            # gate the S->PV PSUM pool handover)
                            nc.vector.tensor_copy(s_sb[it][:, rsl], ps[:])
                        if rr == cores - 1:
                            # softmax fires per row tile as soon as its last
                            # chunk lands: exp(it) on ACT and the P^T XBAR
                            # transpose overlap the remaining S matmuls
                            nc.vector.tensor_scalar_mul(
                                bias_t[it][:], m_t[it][:], -scale
                            )
                            nc.scalar.activation(
                                p_sb[it][:], s_sb[it][:],
                                mybir.ActivationFunctionType.Exp,
                                bias=bias_t[it][:], scale=scale,
                                accum_out=ell_t[it][:],
                            )
                            nc.vector.reciprocal(rl_t[it][:], ell_t[it][:])
                            nc.scalar.dma_start_transpose(pt[it][:], p_sb[it][:])
                    if rr == 2:
                        # bounce rides the scalar queue behind rr<=2's copies,
                        # so the V gather starts only once the khi prefetch
                        # has drained; gpsimd emission stays after the K
                        # collective so khi loads never wait on its tick.
                        nc.scalar.dma_start(bv[:], v_loc[:])
                        nc.gpsimd.collective_compute(
                            "AllGather", mybir.AluOpType.bypass, replica_groups=rg,
                            ins=[bounce_v.ap().opt()], outs=[gath_v.ap().opt()],
                        )
                _spsum_cm.__exit__(None, None, None)

                # ---- O = (P @ V) / ell, all 8 PSUM banks, single V pass ----
                _pvpsum_cm = tc.tile_pool(name="pvpsum", bufs=n_it * n_eh, space="PSUM")
                pvpsum = _pvpsum_cm.__enter__()
                pso = {
                    (it, eh): pvpsum.tile([P, EH], f32, tag="pvps", name="pvps")
                    for it in range(n_it) for eh in range(n_eh)
                }
                for jg in range(n_jg):
                    vc = vchunk.tile([P, JG, dim], f16, tag="vc", name="vc")
                    nc.sync.dma_start(vc[:], gv[jg])
                    last = jg == n_jg - 1
                    for it in range(n_it):
                        for eh in range(n_eh):
                            esl = slice(eh * EH, (eh + 1) * EH)
                            for jj in range(JG):
                                nc.tensor.matmul(
                                    pso[(it, eh)][:],
                                    pt[it][:, jg * JG + jj],
                                    vc[:, jj, esl],
                                    start=(jg == 0 and jj == 0),
                                    stop=(last and jj == JG - 1),
                                )
                        if last:
                            # scale + store this row tile while the PE is
                            # still accumulating the remaining row tiles
                            o_sb = opool.tile([P, dim], f32, tag="o", name="o")
                            for eh in range(n_eh):
                                esl = slice(eh * EH, (eh + 1) * EH)
                                nc.vector.tensor_scalar_mul(
                                    o_sb[:, esl], pso[(it, eh)][:], rl_t[it][:]
                                )
                            nc.scalar.dma_start(
                                out_ext[it * P:(it + 1) * P, :], o_sb[:]
                            )
                _pvpsum_cm.__exit__(None, None, None)

    return nc


_CACHE = {}
RUN_KW = {}


def _get_nc():
    if "nc" not in _CACHE:
        _CACHE["nc"] = build_attention()
    return _CACHE["nc"]


def kernel(**inputs):
    from concourse.bass_utils import run_bass_kernel_spmd

    q = np.ascontiguousarray(np.asarray(inputs["q"], dtype=np.float32))
    k = np.ascontiguousarray(np.asarray(inputs["k"], dtype=np.float32))
    v = np.ascontiguousarray(np.asarray(inputs["v"], dtype=np.float32))
    W_q = np.ascontiguousarray(np.asarray(inputs["W_q"], dtype=np.float32))
    W_k = np.ascontiguousarray(np.asarray(inputs["W_k"], dtype=np.float32))
    W_v = np.ascontiguousarray(np.asarray(inputs["W_v"], dtype=np.float32))

    sh = N_Q // CORES
    in_maps = []
    for r in range(CORES):
        sl = slice(r * sh, (r + 1) * sh)
        in_maps.append({
            "q": q[sl], "k": k[sl], "v": v[sl],
            "W_q": W_q, "W_k": W_k, "W_v": W_v,
        })

    nc = _get_nc()
    if not nc.is_finalized():
        nc.finalize()
    res = run_bass_kernel_spmd(nc, in_maps, core_ids=list(range(CORES)), **RUN_KW)
    _CACHE["last_result"] = res
    out = np.concatenate([res.results[r]["out"] for r in range(CORES)], axis=0)
    return out


if __name__ == "__main__":
    import reference

    inputs = {kk: np.asarray(vv) for kk, vv in reference.setup_inputs().items()}
    out = kernel(**inputs)
    print("out shape:", out.shape, out.dtype)


# revision 38
# speedup vs baseline: 1.1328x; 1.0588x over previous
"""Distributed attention layer kernel for 8 TRN2 NeuronCores.

Reference computation (f32):
    Q = q @ W_q; K = k @ W_k; V = v @ W_v
    out = softmax((Q @ K^T)/sqrt(d_k)) @ V

Sharding: rows of q/k/v are split 8 ways (sequence parallel). Each core
projects its own shards, the K^T/V projections are all-gathered (fp16),
and each core computes its 512-row slice of the attention output.

Precision: projections run in f32r (fp32 operands, PE rounds mantissas
to 11 bits, full rate for free-dim >= 256) with f32 PSUM accumulation.
K^T/Q^T/V are downcast to fp16 for the attention matmuls (QK^T and PV
single plain fp16 matmuls, f32 accumulation). Softmax is f32 (ACT exp
with per-row max bias, fused row-sum). Measured end-to-end error vs the
f32 reference: ~8e-3 (gate 2e-2).
"""

import os
import sys

for _p in ("/opt/pypackages", "/opt/trn_rl_repo"):
    if _p not in sys.path:
        sys.path.insert(0, _p)

import numpy as np

N_Q, N_KV, DIM = 4096, 4096, 1024  # D_K = D_V = DIM (square weights)
CORES = 8

P = 128


def build_attention(nq=N_Q, dim=DIM, cores=CORES):
    """Build the per-core Bass graph (SPMD; identical on all cores)."""
    import concourse.bass as bass
    import concourse.mybir as mybir
    from concourse import bacc
    from concourse.masks import make_identity
    from concourse.tile import TileContext

    dt = mybir.dt
    f32, f32r, f16 = dt.float32, dt.float32r, dt.float16

    sh = nq // cores          # rows per core (512)
    n_ct = dim // P           # contraction tiles for projections (8)
    n_dt = dim // P           # d tiles (8)
    n_it = sh // P            # query-row tiles per core (4)
    n_jjt = sh // P           # kv-row tiles per core (4)
    n_jt = nq // P            # total kv j tiles (32)
    JG = 4                    # j-tiles per PV V-chunk
    n_jg = n_jt // JG         # V chunk count (8)
    EH = 512
    n_eh = dim // EH          # 512-wide output column halves (2)
    hd = dim // 2
    nh = n_dt // 2
    scale = 1.0 / float(np.sqrt(dim))

    nc = bacc.Bacc(num_devices=cores)

    # --- external I/O (per core: row shards of q/k/v, full weights) ---
    q_ext = nc.declare_dram_parameter("q", [sh, dim], f32, isOutput=False)
    k_ext = nc.declare_dram_parameter("k", [sh, dim], f32, isOutput=False)
    v_ext = nc.declare_dram_parameter("v", [sh, dim], f32, isOutput=False)
    wq_ext = nc.declare_dram_parameter("W_q", [dim, dim], f32r, isOutput=False)
    wk_ext = nc.declare_dram_parameter("W_k", [dim, dim], f32r, isOutput=False)
    wv_ext = nc.declare_dram_parameter("W_v", [dim, dim], f32r, isOutput=False)
    out_ext = nc.declare_dram_parameter("out", [sh, dim], f32, isOutput=True)

    # --- internal DRAM for collectives ---
    bounce_k = nc.dram_tensor("bounce_k", [dim, sh], f16)
    bounce_v = nc.dram_tensor("bounce_v", [sh, dim], f16)
    gath_k = nc.dram_tensor("gath_k", [cores * dim, sh], f16, addr_space="Shared")
    gath_v = nc.dram_tensor("gath_v", [cores * sh, dim], f16, addr_space="Shared")

    rg = [list(range(cores))]

    with TileContext(nc) as tc:
        with (
            tc.tile_pool(name="const", bufs=1) as constp,
            tc.tile_pool(name="qt", bufs=1) as qtp,
            tc.tile_pool(name="stats", bufs=1) as statp,
        ):
            # NOTE: make_identity/PE-transpose on float32r crashes walrus
            # codegen; transposes run in plain f32 and the psum result is
            # copy-cast (bit-identical) into float32r SBUF tiles.
            ident_f = constp.tile([P, P], f32, tag="idf", name="idf")
            make_identity(nc, ident_f)

            qthi = qtp.tile([P, n_dt, sh], f16, tag="qthi", name="qthi")
            # v_loc outlives the projection pools: its bounce DMA is issued
            # mid-S-phase to delay the V all-gather until the khi chunk
            # loads have drained (avoids DRAM contention with the gather)
            v_loc = qtp.tile([P, sh // P, dim], f16, tag="v_loc", name="v_loc")

            with (
                tc.tile_pool(name="w", bufs=1) as wpool,
                tc.tile_pool(name="iost", bufs=6) as iost,
                tc.tile_pool(name="tin", bufs=2) as tpool,
                tc.tile_pool(name="kvout", bufs=1) as kvout,
                tc.tile_pool(name="tpsum", bufs=4, space="PSUM") as tpsum,
                tc.tile_pool(name="ppsum", bufs=2, space="PSUM") as ppsum,
            ):
                # All bulk loads (inputs + weights) stream in order on the
                # sync (SP) HWDGE queue; the Activation HWDGE queue is kept
                # for small latency-critical transfers (bounce buffers, P^T
                # XBAR transposes, outputs) so their triggers never stall the
                # ACT engine behind megabytes of weight traffic.
                def load_input(x_ext):
                    stgs = []
                    xsrc = x_ext.rearrange("(it p) c -> p it c", p=P)
                    for it in range(sh // P):
                        stg = iost.tile([P, dim], f32, tag="iostg", name="iostg")
                        nc.sync.dma_start(stg[:], xsrc[:, it])
                        stgs.append(stg)
                    return stgs

                wk = wpool.tile([P, n_ct, dim], f32r, tag="wk", name="wk")
                wq = wpool.tile([P, n_ct, dim], f32r, tag="wq", name="wq")
                wv = wpool.tile([P, n_ct, dim], f32r, tag="wv", name="wv")
                wk_src = wk_ext.rearrange("(ct p) d -> p ct d", p=P)
                wq_src = wq_ext.rearrange("(ct p) d -> p ct d", p=P)
                wv_src = wv_ext.rearrange("(ct p) d -> p ct d", p=P)

                k_stg = load_input(k_ext)
                nc.sync.dma_start(wk[:, :, :hd], wk_src[:, :, :hd])
                nc.sync.dma_start(wk[:, :, hd:], wk_src[:, :, hd:])
                q_stg = load_input(q_ext)
                nc.sync.dma_start(wq[:, :, :hd], wq_src[:, :, :hd])
                nc.sync.dma_start(wq[:, :, hd:], wq_src[:, :, hd:])
                nc.sync.dma_start(wv[:, :, :hd], wv_src[:, :, :hd])
                nc.sync.dma_start(wv[:, :, hd:], wv_src[:, :, hd:])

                def transpose_input(stgs, tag):
                    """Transpose a staged [sh, dim] f32 input on the PE into a
                    [c_in=128, ct, row] f32r SBUF tile (copy-cast from psum)."""
                    xt = tpool.tile([P, n_ct, sh], f32r, tag=tag, name=tag)
                    for it, stg in enumerate(stgs):
                        dst = slice(it * P, (it + 1) * P)
                        for ct in range(n_ct):
                            ps = tpsum.tile([P, P], f32, tag="tps", name="tps")
                            nc.tensor.transpose(
                                ps[:], stg[:, ct * P:(ct + 1) * P], ident_f
                            )
                            nc.vector.tensor_copy(xt[:, ct, dst], ps[:])
                    return xt

                # ---- K path first: project K^T, bounce out, all-gather.
                # Single gather: the kernel-entry CC barrier (~45-55us of
                # launch skew) gates the first collective anyway, and Shared
                # DRAM reads starve while any collective is active, so one
                # gather followed by a full-speed khi prefetch beats split
                # gathers whose chunk reads crawl under the second one. ----
                kt = transpose_input(k_stg, "xt")
                kt_loc = kvout.tile([P, n_dt, sh], f16, tag="kt_loc", name="kt_loc")
                bk = bounce_k.rearrange("(dtt p) jj -> p dtt jj", p=P)
                for dtt in range(n_dt):
                    ps = ppsum.tile([P, sh], f32, tag="pps", name="pps")
                    dsl = slice(dtt * P, (dtt + 1) * P)
                    for ct in range(n_ct):
                        nc.tensor.matmul(
                            ps[:], wk[:, ct, dsl], kt[:, ct],
                            start=(ct == 0), stop=(ct == n_ct - 1),
                        )
                    nc.scalar.copy(kt_loc[:, dtt], ps[:])
                nc.scalar.dma_start(bk[:], kt_loc[:])
                nc.gpsimd.collective_compute(
                    "AllGather", mybir.AluOpType.bypass, replica_groups=rg,
                    ins=[bounce_k.ap().opt()], outs=[gath_k.ap().opt()],
                )

                # ---- Q path (local only): project Q^T, downcast to fp16 ----
                qt = transpose_input(q_stg, "xt")
                for dtt in range(n_dt):
                    ps = ppsum.tile([P, sh], f32, tag="pps", name="pps")
                    dsl = slice(dtt * P, (dtt + 1) * P)
                    for ct in range(n_ct):
                        nc.tensor.matmul(
                            ps[:], wq[:, ct, dsl], qt[:, ct],
                            start=(ct == 0), stop=(ct == n_ct - 1),
                        )
                    nc.scalar.copy(qthi[:, dtt], ps[:])

                # ---- V path: project V shard, downcast ----
                v_stg = load_input(v_ext)
                vt = transpose_input(v_stg, "xt")
                for jjt in range(n_jjt):
                    jsl = slice(jjt * P, (jjt + 1) * P)
                    for eh in range(n_eh):
                        ps = ppsum.tile([P, EH], f32, tag="ppsv", name="ppsv")
                        esl = slice(eh * EH, (eh + 1) * EH)
                        for ct in range(n_ct):
                            nc.tensor.matmul(
                                ps[:], vt[:, ct, jsl], wv[:, ct, esl],
                                start=(ct == 0), stop=(ct == n_ct - 1),
                            )
                        nc.scalar.copy(v_loc[:, jjt, esl], ps[:])

            # ================= attention phase =================
            m_t = [statp.tile([P, 1], f32, tag=f"m{it}", name=f"m{it}") for it in range(n_it)]
            tmpmax = statp.tile([P, 1], f32, tag="tmpmax", name="tmpmax")
            bias_t = [statp.tile([P, 1], f32, tag=f"b{it}", name=f"b{it}") for it in range(n_it)]
            ell_t = [statp.tile([P, 1], f32, tag=f"l{it}", name=f"l{it}") for it in range(n_it)]
            rl_t = [statp.tile([P, 1], f32, tag=f"r{it}", name=f"r{it}") for it in range(n_it)]

            gk = gath_k.rearrange("(r dtt p) jj -> r p dtt jj", r=cores, p=P)
            gv = gath_v.rearrange("(jg jj p) e -> jg p jj e", jj=JG, p=P)
            bv = bounce_v.rearrange("(jjt p) e -> p jjt e", p=P)

            with (
                tc.tile_pool(name="schunk", bufs=5) as schunk,
                tc.tile_pool(name="srow", bufs=n_it) as srow,
                tc.tile_pool(name="prow", bufs=2) as prow,
                tc.tile_pool(name="ptp", bufs=1) as ptp,
                tc.tile_pool(name="vchunk", bufs=3) as vchunk,
                tc.tile_pool(name="opool", bufs=2) as opool,
            ):
                s_sb = [srow.tile([P, nq], f32, tag="s", name="s") for _ in range(n_it)]

                # ---- scores: all khi chunk loads issued upfront (they
                # stream at full bandwidth in the collective-free window
                # right after the K gather), then S with running row max.
                # The V gather is released only after rr==2 so it does not
                # starve the tail of the khi prefetch. ----
                khis = []
                for rr in range(cores):
                    khi = schunk.tile([P, n_dt, sh], f16, tag="khi", name="khi")
                    # two half-loads per chunk: the first 4 dtt matmuls start
                    # as soon as half the bytes land
                    nc.sync.dma_start(khi[:, :nh], gk[rr][:, :nh])
                    nc.sync.dma_start(khi[:, nh:], gk[rr][:, nh:])
                    khis.append(khi)

                p_sb = [prow.tile([P, nq], f16, tag="p", name="p") for _ in range(n_it)]
                pt = [
                    ptp.tile([P, n_jt, P], f16, tag=f"pt{it}", name=f"pt{it}")
                    for it in range(n_it)
                ]

                _spsum_cm = tc.tile_pool(name="spsum", bufs=6, space="PSUM")
                spsum = _spsum_cm.__enter__()
                for rr in range(cores):
                    rsl = slice(rr * sh, (rr + 1) * sh)
                    for it in range(n_it):
                        isl = slice(it * P, (it + 1) * P)
                        ps = spsum.tile([P, sh], f32, tag="sps", name="sps")
                        for dtt in range(n_dt):
                            nc.tensor.matmul(
                                ps[:], qthi[:, dtt, isl], khis[rr][:, dtt],
                                start=(dtt == 0), stop=(dtt == n_dt - 1),
                            )
                        if rr == 0:
                            nc.vector.reduce_max(
                                m_t[it][:], ps[:], axis=mybir.AxisListType.X
                            )
                        else:
                            nc.vector.reduce_max(
                                tmpmax[:], ps[:], axis=mybir.AxisListType.X
                            )
                            nc.vector.tensor_max(m_t[it][:], m_t[it][:], tmpmax[:])
                        if rr < cores - 1:
                            nc.scalar.copy(s_sb[it][:, rsl], ps[:])
                        else:
                            # last chunk's copies go on the vector engine so
                            # the inline exps below don't delay them (they
                            # gate the S->PV PSUM pool handover)
                            nc.vector.tensor_copy(s_sb[it][:, rsl], ps[:])
                        if rr == cores - 1:
                            # softmax fires per row tile as soon as its last
                            # chunk lands: exp(it) on ACT and the P^T XBAR
                            # transpose overlap the remaining S matmuls
                            nc.vector.tensor_scalar_mul(
                                bias_t[it][:], m_t[it][:], -scale
                            )
                            nc.scalar.activation(
                                p_sb[it][:], s_sb[it][:],
                                mybir.ActivationFunctionType.Exp,
                                bias=bias_t[it][:], scale=scale,
                                accum_out=ell_t[it][:],
                            )
                            nc.vector.reciprocal(rl_t[it][:], ell_t[it][:])
                            nc.scalar.dma_start_transpose(pt[it][:], p_sb[it][:])
                    if rr == 2:
                        # bounce rides the scalar queue behind rr<=2's copies,
                        # so the V gather starts only once the khi prefetch
                        # has drained; gpsimd emission stays after the K
                        # collective so khi loads never wait on its tick.
                        nc.scalar.dma_start(bv[:], v_loc[:])
                        nc.gpsimd.collective_compute(
                            "AllGather", mybir.AluOpType.bypass, replica_groups=rg,
                            ins=[bounce_v.ap().opt()], outs=[gath_v.ap().opt()],
                        )
                _spsum_cm.__exit__(None, None, None)

                # ---- O = (P @ V) / ell, all 8 PSUM banks, single V pass ----
                _pvpsum_cm = tc.tile_pool(name="pvpsum", bufs=n_it * n_eh, space="PSUM")
                pvpsum = _pvpsum_cm.__enter__()
                pso = {
                    (it, eh): pvpsum.tile([P, EH], f32, tag="pvps", name="pvps")
                    for it in range(n_it) for eh in range(n_eh)
                }
                for jg in range(n_jg):
                    vc = vchunk.tile([P, JG, dim], f16, tag="vc", name="vc")
                    # per-j-tile loads: matmuls on jj consume each quarter as
                    # it lands instead of waiting for the full 2 MB chunk
                    for jj in range(JG):
                        nc.sync.dma_start(vc[:, jj], gv[jg][:, jj])
                    last = jg == n_jg - 1
                    for it in range(n_it):
                        for eh in range(n_eh):
                            esl = slice(eh * EH, (eh + 1) * EH)
                            for jj in range(JG):
                                nc.tensor.matmul(
                                    pso[(it, eh)][:],
                                    pt[it][:, jg * JG + jj],
                                    vc[:, jj, esl],
                                    start=(jg == 0 and jj == 0),
                                    stop=(last and jj == JG - 1),
                                )
                        if last:
                            # scale + store this row tile while the PE is
                            # still accumulating the remaining row tiles
                            o_sb = opool.tile([P, dim], f32, tag="o", name="o")
                            for eh in range(n_eh):
                                esl = slice(eh * EH, (eh + 1) * EH)
                                nc.vector.tensor_scalar_mul(
                                    o_sb[:, esl], pso[(it, eh)][:], rl_t[it][:]
                                )
                            nc.scalar.dma_start(
                                out_ext[it * P:(it + 1) * P, :], o_sb[:]
                            )
                _pvpsum_cm.__exit__(None, None, None)

    return nc


_CACHE = {}
RUN_KW = {}


def _get_nc():
    if "nc" not in _CACHE:
        _CACHE["nc"] = build_attention()
    return _CACHE["nc"]


def kernel(**inputs):
    from concourse.bass_utils import run_bass_kernel_spmd

    q = np.ascontiguousarray(np.asarray(inputs["q"], dtype=np.float32))
    k = np.ascontiguousarray(np.asarray(inputs["k"], dtype=np.float32))
    v = np.ascontiguousarray(np.asarray(inputs["v"], dtype=np.float32))
    W_q = np.ascontiguousarray(np.asarray(inputs["W_q"], dtype=np.float32))
    W_k = np.ascontiguousarray(np.asarray(inputs["W_k"], dtype=np.float32))
    W_v = np.ascontiguousarray(np.asarray(inputs["W_v"], dtype=np.float32))

    sh = N_Q // CORES
    in_maps = []
    for r in range(CORES):
        sl = slice(r * sh, (r + 1) * sh)
        in_maps.append({
            "q": q[sl], "k": k[sl], "v": v[sl],
            "W_q": W_q, "W_k": W_k, "W_v": W_v,
        })

    nc = _get_nc()
    if not nc.is_finalized():
        nc.finalize()
    res = run_bass_kernel_spmd(nc, in_maps, core_ids=list(range(CORES)), **RUN_KW)
    _CACHE["last_result"] = res
    out = np.concatenate([res.results[r]["out"] for r in range(CORES)], axis=0)
    return out


if __name__ == "__main__":
    import reference

    inputs = {kk: np.asarray(vv) for kk, vv in reference.setup_inputs().items()}
    out = kernel(**inputs)
    print("out shape:", out.shape, out.dtype)
